# revision 18
# baseline (speedup 1.0000x reference)
"""Trainium2 Bass kernel for PrettyRRN (8-node-clique GNN message passing).

Raw-bass implementation (manual semaphores; <=1 sync wait per instruction —
the installed walrus rejects instructions carrying multiple sem waits, which
rules out the Tile layer here). DMA completions are consumed only at
self-fenced points of the DMA queue, which keeps the 16-subengine increment
interleaving race-free.

Math restructuring vs the reference:
  - msg MLP layer 1 over edges (i,j):
      h1[:, (g,i,j)] = relu(W1a^T x0_(g,i) + W1b^T x0_(g,j) + dist_gij * w1d + b1)
    as 3 accumulating PSUM matmuls per 512-edge chunk; the (g,i)/(g,j) gathers
    are zero-stride access patterns on x0 (nothing materialized).
  - diagonal j==i (absent from the reference edge list) killed by a -1e9 mask
    row folded into the dist matmul (relu -> exact 0).
  - segment_sum over source i = free-dim reduce of h1 over j; msg layer 2 then
    runs at node granularity (7x less matmul work than per-edge).
  - per-graph logits sum is pulled in front of the (linear) post layer 2.
  - one-hot encodings built on device: K=4 broadcast matmul + is_equal.
All matmuls in float32r (fp32-width operands, 1 cyc/row at N=512).
Pure data parallel across 8 cores (2048 graphs each).
"""
import sys
sys.path.insert(0, "/opt/trn_rl_repo")
import numpy as np

N_NODES = 8
N_AT = 16
H = 128
BS = 16384
NCORES = 8
G = BS // NCORES          # graphs per core
NEG = -1.0e9

_cache = {}


def _build_module(g=G):
    import concourse.bass as bass
    import concourse.mybir as mybir

    F32 = mybir.dt.float32
    F32R = mybir.dt.float32r
    AF = mybir.ActivationFunctionType
    AX = mybir.AxisListType
    OP = mybir.AluOpType

    nc = bass.Bass(trn_type="TRN2")
    dp = lambda n, s: nc.declare_dram_parameter(n, s, F32, isOutput=False)
    dpr = lambda n, s: nc.declare_dram_parameter(n, s, F32R, isOutput=False)
    nn = g * N_NODES
    nw = g // 64              # 64-graph windows
    nq = g // 256             # 256-graph q blocks (also big-chunk count)
    nwb = g // 512            # 512-graph blocks

    posrows_d = dpr("posrows", [2, nn])
    posP_d = dpr("posP", [16, g])
    idx4_d = dpr("idx4", [4, nn])
    Wn = ["preW1", "preW2", "msgW1a", "msgW1b", "msgW2", "postW1a", "postW1b",
          "postW2", "outW1", "outW2"]
    Wsh = {"preW1": [42, H], "preW2": [H, H], "msgW1a": [H, H], "msgW1b": [H, H],
           "msgW2": [H, H], "postW1a": [H, H], "postW1b": [H, H], "postW2": [H, H],
           "outW1": [H, H], "outW2": [H, N_AT]}
    Wd = {n: dpr(n, Wsh[n]) for n in Wn}
    Bn = ["pre_b1", "pre_b2", "msg_b1", "ub2", "post_b1", "post_b2", "out_b1"]
    Bd = {n: dp(n, [H, 1]) for n in Bn}
    Bd["out_b2"] = dp("out_b2", [N_AT, 1])
    sel4_d = dpr("sel4", [4, 40]); iota40_d = dp("iota40", [40, 1])
    I16_d = dpr("I16", [16, 16]); NEGXY_d = dpr("NEGXY", [2, 16])
    VQ_d = dpr("VQ", [16, 64 * nq])
    LQ_d = dpr("LQ", [72, 8 * nq * H])
    MSK_d = dpr("MSK", [8, 2048])
    out_d = nc.declare_dram_parameter("out", [N_AT, g], F32, isOutput=True)

    # ---- static allocations ----
    ctx = []
    def sb(shape, dt):
        cm = nc.sbuf_tensor(shape, dt)
        t = cm.__enter__(); ctx.append(cm)
        return t
    def ps(shape):
        cm = nc.psum_tensor(shape, F32)
        t = cm.__enter__(); ctx.append(cm)
        return t

    x0T = sb([H, nn], F32R)
    DM = sb([72, 2048], F32R)       # rows 8q+i: dist; 64+i: diag masks
    gsum = sb([H, g], F32R)
    LQ = sb([72, 8 * nq * H], F32R)
    VQ = sb([16, 64 * nq], F32R)
    Wt = {n: sb(Wsh[n], F32R) for n in Wn}
    Bt = {n: sb([H, 1], F32) for n in Bn}
    Bt["out_b2"] = sb([N_AT, 1], F32)
    sel4 = sb([4, 40], F32R); iota40 = sb([40, 1], F32)
    I16 = sb([16, 16], F32R); NEGXY = sb([2, 16], F32R)
    posP = sb([16, g], F32R)
    NB = 2
    PRch = [sb([2, 1024], F32R) for _ in range(NB)]   # posrows chunks (2 windows)
    IXch = [sb([4, 1024], F32R) for _ in range(NB)]   # idx4 chunks (2 windows)
    sqc = [sb([16, 512], F32R) for _ in range(NB)]
    XIN = [sb([42, 1024], F32R) for _ in range(NB)]
    x1 = [sb([H, 512], F32R) for _ in range(NB)]
    h1 = [sb([H, 512], F32) for _ in range(3)]
    Sb = [sb([H, 512], F32R) for _ in range(NB)]
    upd = [sb([H, 512], F32R) for _ in range(NB)]
    xP = [sb([H, 512], F32) for _ in range(NB)]
    ysum = sb([H, 512], F32R)
    zt = sb([H, 512], F32R)
    logT = [sb([N_AT, 512], F32) for _ in range(NB)]

    psD = [ps([128, 512]) for _ in range(NB)]
    psE = [ps([H, 512]) for _ in range(3)]
    psS3 = ps([64, 512])
    psS = [psE[0][0:64, :], psE[1][0:64, :], psE[2][0:64, :], psS3[:]]
    psU = psD[1]   # reused after the D/P phases (barrier-separated)
    psP = psD[0]

    # ---- scheduling framework ----
    streams = {"sync": [], "pe": [], "act": [], "dve": [], "pool": []}
    cnt = {"sync": 0, "pe": 0, "act": 0, "dve": 0, "pool": 0}
    INC = {"sync": 16, "pe": 1, "act": 1, "dve": 1, "pool": 1}

    def emit(engine, fn, waits=()):
        ws = [(s, t) for (s, t) in waits if t > 0]
        streams[engine].append((ws, fn))
        cnt[engine] += INC[engine]
        return (engine, cnt[engine])

    def fence_sync():
        # pin the dma sem to its exact cumulative value so compute engines
        # can soundly wait on it
        streams["sync"].append(([("sync", cnt["sync"])], None))
        return ("sync", cnt["sync"])

    def barrier():
        snap = dict(cnt)
        for e in streams:
            for s, t in snap.items():
                if s != e and t > 0:
                    streams[e].append(([(s, t)], None))

    # ---------------- preamble DMAs (concurrent, one fence) ----------------
    pre_loads = [(LQ, LQ_d), (VQ, VQ_d), (sel4, sel4_d), (iota40, iota40_d),
                 (I16, I16_d), (NEGXY, NEGXY_d), (posP, posP_d)]
    pre_loads += [(Wt[n], Wd[n]) for n in Wn]
    pre_loads += [(Bt[n], Bd[n]) for n in Bn + ["out_b2"]]
    for t, d in pre_loads:
        emit("sync", (lambda t=t, d=d: lambda e: e.dma_start(out=t[:], in_=d[:]))())
    emit("sync", lambda e: e.dma_start(out=DM[64:72, :], in_=MSK_d[:]))
    ev_pre = fence_sync()

    # ---------------- phase D: pairwise distances ----------------
    # q-outer / b-inner: PRch chunk q covers windows cc = 4q+b; psS bank b
    # accumulates over q.
    ev_sq_mm3 = {}
    ev_sq_act = {}
    ev_prum = {}          # per chunk: last mm2 using PRch[chunk%2]
    k = 0
    for q in range(nq):
        for hb in range(2):
            chk = 2 * q + hb
            w_pr = [ev_prum.get(chk - NB, ("pe", 0))]
            emit("sync", (lambda chk=chk: lambda e: e.dma_start(
                out=PRch[chk % NB][:],
                in_=posrows_d[:, 1024 * chk:1024 * chk + 1024]))(), w_pr)
            ev_dma = fence_sync()
            for b2 in range(2):
                b = 2 * hb + b2
                g0 = 64 * (4 * q + b)
                bi = k % NB
                w1 = [ev_pre, ev_sq_act.get(k - NB, ("act", 0))]
                emit("pe", (lambda bi=bi, g0=g0: lambda _: nc.tensor.matmul(
                    psD[bi][0:16, :], I16[:],
                    posP[:, g0:g0 + 64].broadcast_to((16, 64, 8)),
                    start=True, stop=False, skip_group_check=True))(), w1)
                ev_prum[chk] = emit("pe", (lambda bi=bi, chk=chk, b2=b2: lambda _:
                    nc.tensor.matmul(
                        psD[bi][0:16, :], NEGXY[:],
                        PRch[chk % NB][:, 512 * b2:512 * b2 + 512],
                        start=False, stop=True, skip_group_check=True))(), [ev_dma])
                w_sq = [("pe", cnt["pe"]), ev_sq_mm3.get(k - NB, ("pe", 0))]
                ev_sq_act[k] = emit("act", (lambda bi=bi: lambda _: nc.scalar.activation(
                    sqc[bi][:], psD[bi][0:16, :], AF.Square))(), w_sq)
                ev_sq_mm3[k] = emit("pe", (lambda bi=bi, b=b, q=q: lambda _: nc.tensor.matmul(
                    psS[b], VQ[:, 64 * q:64 * q + 64], sqc[bi][:],
                    start=(q == 0), stop=(q == nq - 1), skip_group_check=True))(),
                    [("act", cnt["act"])])
                k += 1
    for b in range(4):
        emit("act", (lambda b=b: lambda _: nc.scalar.activation(
            DM[0:64, 512 * b:512 * b + 512], psS[b], AF.Sqrt))(),
            [("pe", cnt["pe"])])

    barrier()

    # ---------------- phase P: pre-MLP ----------------
    ev_pre1 = {}
    ev_iseq = {}
    ev_x1act = {}
    ev_bc = {}            # per w: mm-bcast (last reader of IXch slice)
    for w in range(nw):
        n0 = 512 * w
        bi = w % NB
        ch = w // 2
        cb = ch % NB
        if w % 2 == 0:
            w_ch = [ev_bc.get(2 * (ch - NB) + 1, ("pe", 0)),
                    ev_pre1.get(2 * (ch - NB) + 1, ("pe", 0))]
            emit("sync", (lambda cb=cb, ch=ch: lambda e: e.dma_start(
                out=IXch[cb][:], in_=idx4_d[:, 1024 * ch:1024 * ch + 1024]))(), w_ch)
            emit("sync", (lambda cb=cb, ch=ch: lambda e: e.dma_start(
                out=XIN[cb][40:42, :], in_=posrows_d[:, 1024 * ch:1024 * ch + 1024]))())
            ev_dma = fence_sync()
        sl = slice(512 * (w % 2), 512 * (w % 2) + 512)
        ev_bc[w] = emit("pe", (lambda bi=bi, cb=cb, sl=sl: lambda _: nc.tensor.matmul(
            psD[bi][0:40, :], sel4[:], IXch[cb][:, sl],
            start=True, stop=True))(), [ev_dma, ev_iseq.get(w - NB, ("dve", 0))])
        ev_iseq[w] = emit("dve", (lambda bi=bi, cb=cb, sl=sl: lambda _:
            nc.vector.tensor_scalar(
                out=XIN[cb][0:40, sl], in0=psD[bi][0:40, :],
                scalar1=iota40[:], scalar2=None, op0=OP.is_equal))(),
            [("pe", cnt["pe"])])
        ev_pre1[w] = emit("pe", (lambda cb=cb, sl=sl: lambda _: nc.tensor.matmul(
            psE[0][:], Wt["preW1"][:], XIN[cb][:, sl], start=True, stop=True))(),
            [("dve", cnt["dve"]), ev_x1act.get(w - 1, ("act", 0))])
        ev_x1act[w] = emit("act", (lambda bi=bi: lambda _: nc.scalar.activation(
            x1[bi][:], psE[0][:], AF.Relu, bias=Bt["pre_b1"][:]))(),
            [("pe", cnt["pe"])])
        emit("pe", (lambda bi=bi: lambda _: nc.tensor.matmul(
            psE[1][:], Wt["preW2"][:], x1[bi][:], start=True, stop=True))(),
            [("act", cnt["act"])])
        emit("act", (lambda n0=n0: lambda _: nc.scalar.activation(
            x0T[:, n0:n0 + 512], psE[1][:], AF.Identity,
            bias=Bt["pre_b2"][:]))(), [("pe", cnt["pe"])])

    barrier()

    # ---------------- phase E: message passing ----------------
    ev_h1_act = {}
    ev_red = {}
    ev_msgL2 = {}
    ev_updact = {}
    ev_posta = {}
    ev_xPact = {}
    ev_gadd = {}
    kk = 0
    for i in range(8):
        for wb in range(nwb):
            blk = i * nwb + wb
            sbi = blk % NB
            for ws in range(8):
                w = 8 * wb + ws
                q, bq = w // 4, w % 4
                g0 = 64 * w
                eb = kk % 3
                s0 = g0 * 8 + i
                w_a = [ev_h1_act.get(kk - 3, ("act", 0))]
                emit("pe", (lambda eb=eb, s0=s0: lambda _: nc.tensor.matmul(
                    psE[eb][:], Wt["msgW1a"][:],
                    x0T[:, s0:s0 + 505:8].broadcast_to((H, 64, 8)),
                    start=True, stop=False))(), w_a)
                emit("pe", (lambda eb=eb, g0=g0: lambda _: nc.tensor.matmul(
                    psE[eb][:], Wt["msgW1b"][:],
                    x0T[:, g0 * 8:(g0 + 64) * 8].rearrange("p (g j) -> p g j", j=8),
                    start=False, stop=False))())
                c0 = H * (8 * q + i)
                emit("pe", (lambda eb=eb, c0=c0, bq=bq: lambda _: nc.tensor.matmul(
                    psE[eb][:], LQ[:, c0:c0 + H], DM[:, 512 * bq:512 * bq + 512],
                    start=False, stop=True))())
                w_h1 = [("pe", cnt["pe"]), ev_red.get(kk - 3, ("dve", 0))]
                ev_h1_act[kk] = emit("act", (lambda eb=eb: lambda _:
                    nc.scalar.activation(h1[eb][:], psE[eb][:], AF.Relu,
                                         bias=Bt["msg_b1"][:]))(), w_h1)
                w_red = [("act", cnt["act"])]
                if ws == 0:
                    w_red.append(ev_msgL2.get(blk - NB, ("pe", 0)))
                def red_fn(eb=eb, sbi=sbi, ws=ws):
                    def f(_):
                        with nc.allow_low_precision(reason="f32r S-reduce"):
                            return nc.vector.tensor_reduce(
                                Sb[sbi][:, 64 * ws:64 * ws + 64],
                                h1[eb][:].rearrange("p (g j) -> p g j", j=8),
                                axis=AX.X, op=OP.add)
                    return f
                ev_red[kk] = emit("dve", red_fn(), w_red)
                kk += 1
            gb0 = 512 * wb
            ib = blk % NB
            w_l2 = [("dve", cnt["dve"]), ev_updact.get(blk - 1, ("act", 0))]
            ev_msgL2[blk] = emit("pe", (lambda sbi=sbi: lambda _:
                nc.tensor.matmul(psU[:], Wt["msgW2"][:], Sb[sbi][:],
                                 start=True, stop=True))(), w_l2)
            w_upd = [("pe", cnt["pe"]), ev_posta.get(blk - NB, ("pe", 0))]
            ev_updact[blk] = emit("act", (lambda ib=ib: lambda _:
                nc.scalar.activation(upd[ib][:], psU[:], AF.Identity,
                                     bias=Bt["ub2"][:]))(), w_upd)
            w_pa = [("act", cnt["act"]), ev_xPact.get(blk - 1, ("act", 0))]
            emit("pe", (lambda ib=ib: lambda _: nc.tensor.matmul(
                psP[:], Wt["postW1a"][:], upd[ib][:], start=True, stop=False))(),
                w_pa)
            sp = gb0 * 8 + i
            ev_posta[blk] = emit("pe", (lambda sp=sp: lambda _:
                nc.tensor.matmul(psP[:], Wt["postW1b"][:],
                                 x0T[:, sp:sp + 4089:8], start=False, stop=True))())
            w_xp = [("pe", cnt["pe"]), ev_gadd.get(blk - NB, ("pool", 0))]
            ev_xPact[blk] = emit("act", (lambda ib=ib: lambda _:
                nc.scalar.activation(xP[ib][:], psP[:], AF.Relu,
                                     bias=Bt["post_b1"][:]))(), w_xp)
            w_g = [("act", cnt["act"])]
            if i > 0:
                w_g.append(ev_gadd[blk - nwb])
            if i == 0:
                ev_gadd[blk] = emit("pool", (lambda ib=ib, gb0=gb0: lambda _:
                    nc.gpsimd.tensor_copy(out=gsum[:, gb0:gb0 + 512],
                                          in_=xP[ib][:]))(), w_g)
            else:
                ev_gadd[blk] = emit("pool", (lambda ib=ib, gb0=gb0: lambda _:
                    nc.gpsimd.tensor_tensor(out=gsum[:, gb0:gb0 + 512],
                                            in0=gsum[:, gb0:gb0 + 512].bitcast(F32),
                                            in1=xP[ib][:], op=OP.add))(), w_g)

    barrier()

    # ---------------- phase O: readout ----------------
    ev_log = {}
    ev_outdma = {}
    for v in range(nwb):
        c0 = 512 * v
        bi = v % NB
        emit("pe", (lambda c0=c0: lambda _: nc.tensor.matmul(
            psU[:], Wt["postW2"][:], gsum[:, c0:c0 + 512],
            start=True, stop=True))(), [ev_log.get(v - 1, ("act", 0))])
        emit("act", lambda _: nc.scalar.activation(
            ysum[:], psU[:], AF.Identity, bias=Bt["post_b2"][:]),
            [("pe", cnt["pe"])])
        emit("pe", lambda _: nc.tensor.matmul(
            psP[:], Wt["outW1"][:], ysum[:], start=True, stop=True),
            [("act", cnt["act"])])
        emit("act", lambda _: nc.scalar.activation(
            zt[:], psP[:], AF.Relu, bias=Bt["out_b1"][:]),
            [("pe", cnt["pe"])])
        emit("pe", lambda _: nc.tensor.matmul(
            psU[0:N_AT, :], Wt["outW2"][:], zt[:], start=True, stop=True),
            [("act", cnt["act"])])
        ev_log[v] = emit("act", (lambda bi=bi: lambda _: nc.scalar.activation(
            logT[bi][:], psU[0:N_AT, :], AF.Identity, bias=Bt["out_b2"][:]))(),
            [("pe", cnt["pe"]), ev_outdma.get(v - NB, ("sync", 0))])
        emit("sync", (lambda bi=bi, c0=c0: lambda e: e.dma_start(
            out=out_d[:, c0:c0 + 512], in_=logT[bi][:]))(), [("act", cnt["act"])])
        ev_outdma[v] = fence_sync()

    # ---------------- lower to engine blocks ----------------
    sem_cms = {n: nc.semaphore(f"s_{n}") for n in streams}
    sems = {}
    for n, cm in sem_cms.items():
        sems[n] = cm.__enter__(); ctx.append(cm)

    block_cm = nc.Block()
    block = block_cm.__enter__()

    def make_body(name):
        def body(eng):
            hwm = {n2: 0 for n2 in streams}
            my_sem = sems[name]
            inc = INC[name]
            for ws, fn in streams[name]:
                for s, t in ws:
                    if t > hwm[s] or (s == name and t == hwm[s]):
                        eng.wait_ge(sems[s], t)
                        hwm[s] = t
                if fn is not None:
                    inst = fn(eng)
                    inst.then_inc(my_sem, inc)
        return body

    block.sync(make_body("sync"))
    block.tensor(make_body("pe"))
    block.scalar(make_body("act"))
    block.vector(make_body("dve"))
    block.gpsimd(make_body("pool"))

    block_cm.__exit__(None, None, None)
    for cm in reversed(ctx):
        cm.__exit__(None, None, None)
    return nc


def _host_consts(weights, nq=G // 256):
    f32 = np.float32
    w1d = weights["msg_W1"][256].astype(f32)
    LQ = np.zeros((72, 8 * nq * H), f32)
    for q in range(nq):
        for i in range(8):
            c = H * (8 * q + i)
            LQ[8 * q + i, c:c + H] = w1d
            LQ[64 + i, c:c + H] = NEG
    VQ = np.zeros((16, 64 * nq), f32)
    for q in range(nq):
        for i in range(8):
            VQ[2 * i, 64 * q + 8 * q + i] = 1.0
            VQ[2 * i + 1, 64 * q + 8 * q + i] = 1.0
    MSK = np.zeros((8, 2048), f32)
    for i in range(8):
        MSK[i, i::8] = 1.0
    sel4 = np.zeros((4, 40), f32)
    sel4[0, 0:8] = 1; sel4[1, 8:16] = 1; sel4[2, 16:32] = 1; sel4[3, 32:40] = 1
    iota40 = np.concatenate([np.arange(8), np.arange(8), np.arange(16),
                             np.arange(8)]).astype(f32).reshape(40, 1)
    I16 = np.eye(16, dtype=f32)
    NEGXY = -np.tile(np.eye(2, dtype=f32), 8)
    col = lambda n: np.ascontiguousarray(weights[n].astype(f32))
    shared = {
        "preW1": np.ascontiguousarray(
            np.concatenate([weights["pre_W1"][2:42], weights["pre_W1"][0:2]]).astype(f32)),
        "preW2": col("pre_W2"),
        "msgW1a": np.ascontiguousarray(weights["msg_W1"][0:128].astype(f32)),
        "msgW1b": np.ascontiguousarray(weights["msg_W1"][128:256].astype(f32)),
        "msgW2": col("msg_W2"),
        "postW1a": np.ascontiguousarray(weights["post_W1"][0:128].astype(f32)),
        "postW1b": np.ascontiguousarray(weights["post_W1"][128:256].astype(f32)),
        "postW2": col("post_W2"), "outW1": col("out_W1"), "outW2": col("out_W2"),
        "pre_b1": col("pre_b1").reshape(H, 1), "pre_b2": col("pre_b2").reshape(H, 1),
        "msg_b1": col("msg_b1").reshape(H, 1),
        "ub2": (7.0 * weights["msg_b2"]).astype(f32).reshape(H, 1),
        "post_b1": col("post_b1").reshape(H, 1),
        "post_b2": (8.0 * weights["post_b2"]).astype(f32).reshape(H, 1),
        "out_b1": col("out_b1").reshape(H, 1), "out_b2": col("out_b2").reshape(N_AT, 1),
        "sel4": sel4, "iota40": iota40, "I16": I16, "NEGXY": NEGXY,
        "VQ": VQ, "LQ": LQ, "MSK": MSK,
    }
    return shared


def kernel(**inputs):
    from concourse.bass_utils import run_bass_kernel_spmd

    if "nc" not in _cache:
        _cache["nc"] = _build_module()
    nc = _cache["nc"]

    f32 = np.float32
    wnames = ["pre_W1", "pre_b1", "pre_W2", "pre_b2", "msg_W1", "msg_b1", "msg_W2",
              "msg_b2", "post_W1", "post_b1", "post_W2", "post_b2", "out_W1",
              "out_b1", "out_W2", "out_b2"]
    shared = _host_consts({n: np.asarray(inputs[n]) for n in wnames})

    anchors = np.asarray(inputs["anchors"]).astype(f32)
    n_jumps = np.asarray(inputs["n_jumps"]).astype(f32)
    positions = np.asarray(inputs["positions"]).astype(f32)
    colors = np.asarray(inputs["colors"]).astype(f32)
    markers = np.asarray(inputs["markers"]).astype(f32) - 8.0

    in_maps = []
    for c in range(NCORES):
        sl = slice(c * G, (c + 1) * G)
        pos = positions[sl]
        posrows = np.ascontiguousarray(pos.reshape(G * N_NODES, 2).T)
        posP = np.ascontiguousarray(pos.transpose(1, 2, 0).reshape(16, G))
        idx4 = np.ascontiguousarray(np.stack(
            [colors[sl].reshape(-1), markers[sl].reshape(-1),
             np.repeat(anchors[sl], N_NODES),
             np.repeat(n_jumps[sl], N_NODES)]).astype(f32))
        m = {"posrows": posrows, "posP": posP, "idx4": idx4}
        m.update(shared)
        in_maps.append(m)

    res = run_bass_kernel_spmd(nc, in_maps, list(range(NCORES)))
    _cache["last_results"] = res
    outs = [res.results[c]["out"] for c in range(NCORES)]
    return np.concatenate(outs, axis=1).T.copy()
